# revision 5
# baseline (speedup 1.0000x reference)
"""Single-head cross-attention (layernorm + QKV proj + softmax(QK^T)V) on 8 NeuronCores.

Sharding: data-parallel over batch B=8, one batch element per core.

v2 design (vs the DMA-xbar-transpose baseline):
  * Host ships BOTH natural fp32 x (for stats) and pre-transposed bf16 x^T
    (for matmuls) — no on-device activation transposes at all.
  * Layernorm is folded into the projections:
      q^T[e,i] = rstd_i * (W^T x^T_raw + colsum_w ⊗ (-mu) + b ⊗ inv_rstd)[e,i]
    The two rank-1 corrections ride a single K=2 matmul appended to each
    PSUM accumulation group; the per-token rstd scale is applied at PSUM
    evacuation (DVE tensor_mul with a broadcast rstd row for q/k in
    transposed layout; ScalarE per-partition scale for v in natural layout).
  * Stats rows (free-dim layout) are produced by a tiny PE transpose of the
    per-chunk stats columns; rstd broadcast goes through a DRAM round-trip.
  * Scores K^T·Q run in fp8(e4m3) with DoubleRow perf mode (2 d-blocks per
    matmul) — rel err ~1.2e-2 (gate 2e-2), everything else bf16/fp32.
  * attn·V and the Z (softmax denominator) matmuls as in the baseline.
"""

import os
from contextlib import ExitStack

import numpy as np
import ml_dtypes

import concourse.bass as bass
import concourse.bacc as bacc
import concourse.mybir as mybir
import concourse.tile as tile
from concourse.bass import ts, ds
from concourse.bass_utils import run_bass_kernel_spmd

BF16 = mybir.dt.bfloat16
F32 = mybir.dt.float32
F8 = mybir.dt.float8e4

B, T, D = 8, 2048, 1024
EPS = 1e-5
SCALE = float(D) ** -0.5
P = 128
N_IT = T // P          # 16 token tiles of 128
N_DB = D // P          # 8 d-blocks of 128
N_EB = D // P          # 8 e-blocks of 128
N_IC = T // 512        # 4 token chunks of 512
N_EC = D // 512        # 2 e chunks of 512

AF = mybir.ActivationFunctionType


def build_module() -> bass.Bass:
    nc = bacc.Bacc("TRN2", target_bir_lowering=False)

    x_nat = {}
    x_tr = {}
    w_d = {}
    cs_d = {}
    b_d = {}
    for nm in ("t", "k", "v"):
        x_nat[nm] = nc.dram_tensor(f"x_{nm}", [T, D], BF16, kind="ExternalInput")
        x_tr[nm] = nc.dram_tensor(f"xt_{nm}", [D, T], BF16, kind="ExternalInput")
        w_d[nm] = nc.dram_tensor(f"w_{nm}", [D, D], BF16, kind="ExternalInput")  # [d, e]
        cs_d[nm] = nc.dram_tensor(f"cs_{nm}", [D], BF16, kind="ExternalInput")
        b_d[nm] = nc.dram_tensor(f"b_{nm}", [D], BF16, kind="ExternalInput")
    ident_d = nc.dram_tensor("ident", [P, P], F32, kind="ExternalInput")
    out = nc.dram_tensor("out", [T, D], F32, kind="ExternalOutput")
    rstd_dram = {nm: nc.dram_tensor(f"rstd_d_{nm}", [T], BF16, kind="Internal")
                 for nm in ("t", "k")}

    with tile.TileContext(nc) as tc, ExitStack() as ctx:
        const = ctx.enter_context(tc.tile_pool(name="const", bufs=1))
        qkv = ctx.enter_context(tc.tile_pool(name="qkv", bufs=1))
        mm_ps = ctx.enter_context(tc.tile_pool(name="mm_ps", bufs=4, space="PSUM"))

        # ---- constants ----
        eps_t = const.tile([P, 1], F32)
        nc.vector.memset(eps_t, EPS)
        ones_t = const.tile([P, 1], BF16)
        nc.vector.memset(ones_t, 1.0)
        ident = const.tile([P, P], F32)
        nc.sync.dma_start(out=ident, in_=ident_d[:, :])

        # corr_w[X]: partition0 = colsum(w_eff), partition1 = folded bias.
        # bf16: fp32 matmuls run at 1/4 rate (2 half-speed passes) and the
        # correction terms are small relative to q/k/v (~3% magnitude).
        corr_w = {}
        for nm in ("t", "k", "v"):
            cw = qkv.tile([2, D], BF16, name=f"corr_w_{nm}")
            nc.sync.dma_start(out=cw[ds(0, 1), :], in_=cs_d[nm][:].unsqueeze(0))
            nc.sync.dma_start(out=cw[ds(1, 1), :], in_=b_d[nm][:].unsqueeze(0))
            corr_w[nm] = cw

        # ---- persistent projection outputs ----
        qT = qkv.tile([P, N_EB, T], F8)          # qT[p, eb, i] = q[i, eb*128+p]
        kT = qkv.tile([P, N_EB, T], F8)
        v_sb = qkv.tile([P, N_IT, D], BF16)      # v[p, jt, e] = v[jt*128+p, e]
        corr_rows = {nm: qkv.tile([2, T], BF16, name=f"corr_rows_{nm}")
                     for nm in ("t", "k", "v")}  # p0 = -mu, p1 = sqrt(var+eps)
        rstd_bc = {nm: qkv.tile([P, T], BF16, name=f"rstd_bc_{nm}")
                   for nm in ("t", "k")}         # rstd broadcast along partitions
        rstd_cols_v = qkv.tile([P, N_IT], F32)   # v rstd, natural col layout

        with tc.tile_pool(name="stats_ps", bufs=2, space="PSUM") as stats_ps, \
             tc.tile_pool(name="proj_phase", bufs=1) as pp:

            def stats_dve(nm, ic):
                """DVE/ACT part of the stats chain for one token chunk."""
                scols = pp.tile([P, 12], F32, tag="scols", bufs=3,
                                name=f"scols_{nm}_{ic}")
                for tl in range(4):
                    it = 4 * ic + tl
                    x_raw = pp.tile([P, D], BF16, tag="x_raw", bufs=3,
                                    name=f"x_raw_{nm}_{it}")
                    nc.gpsimd.dma_start(out=x_raw, in_=x_nat[nm][ts(it, P), :])
                    st6 = pp.tile([P, 2, 6], F32, tag="st6", bufs=3,
                                  name=f"st6_{nm}_{it}")
                    for sb in range(2):
                        nc.vector.bn_stats(out=st6[:, sb, :], in_=x_raw[:, ts(sb, 512)])
                    mv = pp.tile([P, 2], F32, tag="mv", bufs=3,
                                 name=f"mv_{nm}_{it}")
                    nc.vector.bn_aggr(out=mv, in_=st6)
                    # col 0..3: -mu ; col 4..7: sqrt(var+eps) ; col 8..11: rstd
                    nc.scalar.activation(out=scols[:, tl:tl + 1], in_=mv[:, 0:1],
                                         func=AF.Copy, scale=-1.0)
                    nc.scalar.activation(out=scols[:, 4 + tl:5 + tl], in_=mv[:, 1:2],
                                         func=AF.Sqrt, bias=eps_t)
                    rstd_out = (rstd_cols_v[:, it:it + 1] if nm == "v"
                                else scols[:, 8 + tl:9 + tl])
                    nc.vector.reciprocal(out=rstd_out,
                                         in_=scols[:, 4 + tl:5 + tl])
                return scols

            def stats_finish(nm, ic, scols):
                """PE transpose + row assembly; emitted just before the consumer."""
                ncol = 12 if nm in ("t", "k") else 8
                st_ps = stats_ps.tile([12, P], F32, tag="stats",
                                      name=f"stps_{nm}_{ic}")
                nc.tensor.transpose(st_ps[ds(0, ncol), :], in_=scols[:, 0:ncol],
                                    identity=ident)
                st_sb = pp.tile([12, P], BF16, tag="st_sb", bufs=2,
                                name=f"stsb_{nm}_{ic}")
                nc.vector.tensor_copy(out=st_sb[ds(0, ncol), :],
                                      in_=st_ps[ds(0, ncol), :])
                nc.sync.dma_start(out=corr_rows[nm][ds(0, 1), ts(ic, 512)],
                                  in_=st_sb[ds(0, 4), :])
                nc.sync.dma_start(out=corr_rows[nm][ds(1, 1), ts(ic, 512)],
                                  in_=st_sb[ds(4, 4), :])
                if nm in ("t", "k"):
                    nc.sync.dma_start(out=rstd_dram[nm][ts(ic, 512)],
                                      in_=st_sb[ds(8, 4), :])
                    rd = rstd_dram[nm][ts(ic, 512)]
                    nc.sync.dma_start(
                        out=rstd_bc[nm][:, ts(ic, 512)],
                        in_=bass.AP(tensor=rd.tensor, offset=rd.offset,
                                    ap=[[0, P]] + list(rd.ap)))

            def stats_chunk(nm, ic):
                stats_finish(nm, ic, stats_dve(nm, ic))

            def load_w(nm):
                w_all = pp.tile([P, N_DB, D], BF16, tag="w_all", bufs=2,
                                name=f"w_all_{nm}")
                # two DMAs so the first e-half is usable before the second lands
                for h in range(2):
                    nc.sync.dma_start(
                        out=w_all[:, :, ts(h, 512)],
                        in_=w_d[nm][:, ts(h, 512)].rearrange("(a p) e -> p a e", p=P))
                return w_all

            def load_xt(nm, ic):
                xt_sb = pp.tile([P, N_DB, 512], BF16, tag="xt", bufs=3,
                                name=f"xt_{nm}_{ic}")
                nc.sync.dma_start(
                    out=xt_sb,
                    in_=x_tr[nm][:, ts(ic, 512)].rearrange("(a p) t -> p a t", p=P))
                return xt_sb

            def proj_qk(nm, dstT, w_all, xt_sb, ic):
                """dstT[:, eb, chunk ic] in fp8, LN+bias folded."""
                for eb in range(N_EB):
                    ps = mm_ps.tile([P, 512], F32, tag="mm",
                                    name=f"ps_{nm}_{ic}_{eb}")
                    for db in range(N_DB):
                        nc.tensor.matmul(ps, lhsT=w_all[:, db, ts(eb, P)],
                                         rhs=xt_sb[:, db, :],
                                         start=(db == 0), stop=False)
                    nc.tensor.matmul(ps, lhsT=corr_w[nm][:, ts(eb, P)],
                                     rhs=corr_rows[nm][:, ts(ic, 512)],
                                     start=False, stop=True)
                    nc.vector.tensor_mul(out=dstT[:, eb, ts(ic, 512)], in0=ps,
                                         in1=rstd_bc[nm][:, ts(ic, 512)])

            def proj_v(w_all, xt_sb, ic):
                for ec in range(N_EC):
                    for ml in range(4):
                        m = 4 * ic + ml
                        ps = mm_ps.tile([P, 512], F32, tag="mm",
                                        name=f"ps_v_{m}_{ec}")
                        for db in range(N_DB):
                            nc.tensor.matmul(ps,
                                             lhsT=xt_sb[:, db, ds(ml * P, P)],
                                             rhs=w_all[:, db, ts(ec, 512)],
                                             start=(db == 0), stop=False)
                        nc.tensor.matmul(ps, lhsT=corr_rows["v"][:, ts(m, P)],
                                         rhs=corr_w["v"][:, ts(ec, 512)],
                                         start=False, stop=True)
                        nc.scalar.activation(out=v_sb[:, m, ts(ec, 512)], in_=ps,
                                             func=AF.Identity,
                                             scale=rstd_cols_v[:, m:m + 1])

            # Each tensor's stats chain is emitted one projection-phase early
            # so the in-order DVE/PE queues have it ready when the projection
            # needs corr rows (avoids a stall at each phase transition).
            w_k = load_w("k")
            with nc.named_scope("proj_k"):
                scols_k = stats_dve("k", 0)
                for ic in range(N_IC):
                    xt_sb = load_xt("k", ic)
                    stats_finish("k", ic, scols_k)
                    if ic + 1 < N_IC:
                        scols_k = stats_dve("k", ic + 1)
                    proj_qk("k", kT, w_k, xt_sb, ic)
                    stats_chunk("t", ic)
            w_q = load_w("t")
            with nc.named_scope("proj_q"):
                for ic in range(N_IC):
                    xt_sb = load_xt("t", ic)
                    proj_qk("t", qT, w_q, xt_sb, ic)
                    stats_chunk("v", ic)
            w_v = load_w("v")
            with nc.named_scope("proj_v"):
                for ic in range(N_IC):
                    xt_sb = load_xt("v", ic)
                    proj_v(w_v, xt_sb, ic)

        # ---- attention ----
        attv_ps = ctx.enter_context(tc.tile_pool(name="attv_ps", bufs=2, space="PSUM"))
        with tc.tile_pool(name="att", bufs=1) as att:
            for ic in range(N_IC):
                with nc.named_scope(f"scores_{ic}"):
                    aT = att.tile([P, N_IT, 512], BF16, tag="aT", bufs=2,
                                  name=f"aT_{ic}")
                    for jt in range(N_IT):
                        ps = mm_ps.tile([P, 512], F32, tag="mm",
                                        name=f"ps_s_{ic}_{jt}")
                        for ebp in range(N_EB // 2):
                            nc.tensor.matmul(
                                ps, lhsT=kT[:, ds(2 * ebp, 2), ts(jt, P)],
                                rhs=qT[:, ds(2 * ebp, 2), ts(ic, 512)],
                                start=(ebp == 0), stop=(ebp == N_EB // 2 - 1),
                                perf_mode=mybir.MatmulPerfMode.DoubleRow)
                        nc.scalar.activation(
                            out=aT[:, jt, :], in_=ps,
                            func=AF.Exp, scale=SCALE)
                with nc.named_scope(f"attv_{ic}"):
                    for isub in range(4):
                        ou = attv_ps.tile([P, D], F32, tag="ou",
                                          name=f"ou_{ic}_{isub}")
                        zz = mm_ps.tile([P, 1], F32, tag="mm",
                                        name=f"z_{ic}_{isub}")
                        # same-bank runs of 16 accumulating matmuls (bank cycling
                        # between consecutive matmuls forces PE micro-stalls)
                        for ec in range(N_EC):
                            for jt in range(N_IT):
                                nc.tensor.matmul(
                                    ou[:, ts(ec, 512)],
                                    lhsT=aT[:, jt, ts(isub, P)],
                                    rhs=v_sb[:, jt, ts(ec, 512)],
                                    start=(jt == 0), stop=(jt == N_IT - 1))
                        for jt in range(N_IT):
                            nc.tensor.matmul(zz, lhsT=aT[:, jt, ts(isub, P)],
                                             rhs=ones_t,
                                             start=(jt == 0), stop=(jt == N_IT - 1))
                        rz = att.tile([P, 1], F32, tag="rz", bufs=2,
                                      name=f"rz_{ic}_{isub}")
                        nc.vector.reciprocal(out=rz, in_=zz)
                        o_sb = att.tile([P, D], F32, tag="o_sb", bufs=2,
                                        name=f"o_{ic}_{isub}")
                        nc.vector.tensor_scalar_mul(out=o_sb, in0=ou, scalar1=rz)
                        nc.sync.dma_start(out=out[ts(ic * 4 + isub, P), :], in_=o_sb)

    nc.compile()
    return nc


_NC_CACHE = None


def _get_module():
    global _NC_CACHE
    if _NC_CACHE is None:
        _NC_CACHE = build_module()
    return _NC_CACHE


def host_prep(target, source_k, source_v, Wq, bq, Wk, bk, Wv, bv,
              g_t, b_t, g_k, b_k, g_v, b_v):
    """Shared host-side input prep; returns per-core in_maps."""
    bf16 = ml_dtypes.bfloat16
    f32 = np.float32
    Wq = np.asarray(Wq, f32); bq = np.asarray(bq, f32)
    Wk = np.asarray(Wk, f32); bk = np.asarray(bk, f32)
    Wv = np.asarray(Wv, f32); bv = np.asarray(bv, f32)
    g_t = np.asarray(g_t, f32); b_t = np.asarray(b_t, f32)
    g_k = np.asarray(g_k, f32); b_k = np.asarray(b_k, f32)
    g_v = np.asarray(g_v, f32); b_v = np.asarray(b_v, f32)

    # Fold the layernorm affine (g, b) into the projection weights/biases:
    #   LN_affine(x) @ W.T + b  ==  LN_plain(x) @ (W*g).T + (b + W @ b_ln)
    wts = {"t": np.ascontiguousarray((Wq * g_t[None, :]).T).astype(bf16),
           "k": np.ascontiguousarray((Wk * g_k[None, :]).T).astype(bf16),
           "v": np.ascontiguousarray((Wv * g_v[None, :]).T).astype(bf16)}
    bias = {"t": (bq + Wq @ b_t).astype(bf16), "k": (bk + Wk @ b_k).astype(bf16),
            "v": (bv + Wv @ b_v).astype(bf16)}
    csum = {nm: wts[nm].astype(f32).sum(axis=0).astype(bf16) for nm in wts}
    ident = np.eye(P, dtype=f32)

    xs = {"t": np.asarray(target, f32), "k": np.asarray(source_k, f32),
          "v": np.asarray(source_v, f32)}
    in_maps = []
    for b in range(B):
        im = {"ident": ident}
        for nm in ("t", "k", "v"):
            im[f"x_{nm}"] = np.ascontiguousarray(xs[nm][b]).astype(bf16)
            im[f"xt_{nm}"] = np.ascontiguousarray(xs[nm][b].T).astype(bf16)
            im[f"w_{nm}"] = wts[nm]
            im[f"cs_{nm}"] = csum[nm]
            im[f"b_{nm}"] = bias[nm]
        in_maps.append(im)
    return in_maps


def kernel(target, source_k, source_v, Wq, bq, Wk, bk, Wv, bv,
           g_t, b_t, g_k, b_k, g_v, b_v):
    in_maps = host_prep(target, source_k, source_v, Wq, bq, Wk, bk, Wv, bv,
                        g_t, b_t, g_k, b_k, g_v, b_v)
    nc = _get_module()
    res = run_bass_kernel_spmd(nc, in_maps, core_ids=list(range(B)),
                               trace=bool(int(os.environ.get("KERNEL_TRACE", "0"))))
    out = np.stack([res.results[b]["out"] for b in range(B)], axis=0)
    kernel.last_results = res
    return out


# revision 6
# speedup vs baseline: 1.0352x; 1.0352x over previous
"""Single-head cross-attention (layernorm + QKV proj + softmax(QK^T)V) on 8 NeuronCores.

Sharding: data-parallel over batch B=8, one batch element per core.

v2 design (vs the DMA-xbar-transpose baseline):
  * Host ships BOTH natural fp32 x (for stats) and pre-transposed bf16 x^T
    (for matmuls) — no on-device activation transposes at all.
  * Layernorm is folded into the projections:
      q^T[e,i] = rstd_i * (W^T x^T_raw + colsum_w ⊗ (-mu) + b ⊗ inv_rstd)[e,i]
    The two rank-1 corrections ride a single K=2 matmul appended to each
    PSUM accumulation group; the per-token rstd scale is applied at PSUM
    evacuation (DVE tensor_mul with a broadcast rstd row for q/k in
    transposed layout; ScalarE per-partition scale for v in natural layout).
  * Stats rows (free-dim layout) are produced by a tiny PE transpose of the
    per-chunk stats columns; rstd broadcast goes through a DRAM round-trip.
  * Scores K^T·Q run in fp8(e4m3) with DoubleRow perf mode (2 d-blocks per
    matmul) — rel err ~1.2e-2 (gate 2e-2), everything else bf16/fp32.
  * attn·V and the Z (softmax denominator) matmuls as in the baseline.
"""

import os
from contextlib import ExitStack

import numpy as np
import ml_dtypes

import concourse.bass as bass
import concourse.bacc as bacc
import concourse.mybir as mybir
import concourse.tile as tile
from concourse.bass import ts, ds
from concourse.bass_utils import run_bass_kernel_spmd

BF16 = mybir.dt.bfloat16
F32 = mybir.dt.float32
F8 = mybir.dt.float8e4

B, T, D = 8, 2048, 1024
EPS = 1e-5
SCALE = float(D) ** -0.5
P = 128
N_IT = T // P          # 16 token tiles of 128
N_DB = D // P          # 8 d-blocks of 128
N_EB = D // P          # 8 e-blocks of 128
N_IC = T // 512        # 4 token chunks of 512
N_EC = D // 512        # 2 e chunks of 512

AF = mybir.ActivationFunctionType


def build_module() -> bass.Bass:
    nc = bacc.Bacc("TRN2", target_bir_lowering=False)

    x_nat = {}
    x_tr = {}
    w_d = {}
    cs_d = {}
    b_d = {}
    for nm in ("t", "k", "v"):
        x_nat[nm] = nc.dram_tensor(f"x_{nm}", [T, D], BF16, kind="ExternalInput")
        x_tr[nm] = nc.dram_tensor(f"xt_{nm}", [D, T], BF16, kind="ExternalInput")
        w_d[nm] = nc.dram_tensor(f"w_{nm}", [D, D], BF16, kind="ExternalInput")  # [d, e]
        cs_d[nm] = nc.dram_tensor(f"cs_{nm}", [D], BF16, kind="ExternalInput")
        b_d[nm] = nc.dram_tensor(f"b_{nm}", [D], BF16, kind="ExternalInput")
    ident_d = nc.dram_tensor("ident", [P, P], F32, kind="ExternalInput")
    out = nc.dram_tensor("out", [T, D], F32, kind="ExternalOutput")
    rstd_dram = {nm: nc.dram_tensor(f"rstd_d_{nm}", [T], BF16, kind="Internal")
                 for nm in ("t",)}

    with tile.TileContext(nc) as tc, ExitStack() as ctx:
        const = ctx.enter_context(tc.tile_pool(name="const", bufs=1))
        qkv = ctx.enter_context(tc.tile_pool(name="qkv", bufs=1))

        # ---- constants ----
        eps_t = const.tile([P, 1], F32)
        nc.vector.memset(eps_t, EPS)
        epsk_t = const.tile([P, 1], F32)
        nc.vector.memset(epsk_t, EPS / (SCALE * SCALE))
        ones_t = const.tile([P, 1], BF16)
        nc.vector.memset(ones_t, 1.0)
        ident = const.tile([P, P], F32)
        nc.sync.dma_start(out=ident, in_=ident_d[:, :])

        # corr_w[X]: partition0 = colsum(w_eff), partition1 = folded bias.
        # bf16: fp32 matmuls run at 1/4 rate (2 half-speed passes) and the
        # correction terms are small relative to q/k/v (~3% magnitude).
        corr_w = {}
        for nm in ("t", "k", "v"):
            cw = qkv.tile([2, D], BF16, name=f"corr_w_{nm}")
            nc.sync.dma_start(out=cw[ds(0, 1), :], in_=cs_d[nm][:].unsqueeze(0))
            nc.sync.dma_start(out=cw[ds(1, 1), :], in_=b_d[nm][:].unsqueeze(0))
            corr_w[nm] = cw

        # ---- persistent projection outputs ----
        qT = qkv.tile([P, N_EB, T], F8)          # qT[p, eb, i] = q[i, eb*128+p]
        kT = qkv.tile([P, N_EB, T], F8)
        v_sb = qkv.tile([P, N_IT, D], BF16)      # v[p, jt, e] = v[jt*128+p, e]
        corr_rows = {nm: qkv.tile([2, T], BF16, name=f"corr_rows_{nm}")
                     for nm in ("t", "k", "v")}  # p0 = -mu, p1 = sqrt(var+eps)
        rstd_bc = {nm: qkv.tile([P, T], BF16, name=f"rstd_bc_{nm}")
                   for nm in ("t",)}             # rstd broadcast along partitions
        rstd_cols_v = qkv.tile([P, N_IT], F32)   # v rstd, natural col layout
        srstd_cols_k = qkv.tile([P, N_IT], F32)  # SCALE * rstd_k, folded into exp

        with tc.tile_pool(name="stats_ps", bufs=2, space="PSUM") as stats_ps, \
             tc.tile_pool(name="mm_ps", bufs=6, space="PSUM") as mm_ps, \
             tc.tile_pool(name="proj_phase", bufs=1) as pp:

            def stats_dve(nm, ic):
                """DVE/ACT part of the stats chain for one token chunk."""
                scols = pp.tile([P, 12], F32, tag="scols", bufs=3,
                                name=f"scols_{nm}_{ic}")
                for tl in range(4):
                    it = 4 * ic + tl
                    x_raw = pp.tile([P, D], BF16, tag="x_raw", bufs=4,
                                    name=f"x_raw_{nm}_{it}")
                    nc.gpsimd.dma_start(out=x_raw, in_=x_nat[nm][ts(it, P), :])
                    st6 = pp.tile([P, 2, 6], F32, tag="st6", bufs=3,
                                  name=f"st6_{nm}_{it}")
                    for sb in range(2):
                        nc.vector.bn_stats(out=st6[:, sb, :], in_=x_raw[:, ts(sb, 512)])
                    mv = pp.tile([P, 2], F32, tag="mv", bufs=3,
                                 name=f"mv_{nm}_{it}")
                    nc.vector.bn_aggr(out=mv, in_=st6)
                    # col 0..3: -mu ; col 4..7: sqrt(var+eps) ; col 8..11: rstd
                    nc.scalar.activation(out=scols[:, tl:tl + 1], in_=mv[:, 0:1],
                                         func=AF.Copy, scale=-1.0)
                    nc.scalar.activation(out=scols[:, 4 + tl:5 + tl], in_=mv[:, 1:2],
                                         func=AF.Sqrt, bias=eps_t)
                    if nm == "t":
                        # rstd row -> DRAM round-trip -> partition broadcast
                        nc.vector.reciprocal(out=scols[:, 8 + tl:9 + tl],
                                             in_=scols[:, 4 + tl:5 + tl])
                    elif nm == "v":
                        nc.vector.reciprocal(out=rstd_cols_v[:, it:it + 1],
                                             in_=scols[:, 4 + tl:5 + tl])
                    else:
                        # k: exp folds SCALE*rstd_j per partition; build
                        # 1/(SCALE^-1 * sqrt(var+eps)) via a scaled sqrt
                        nc.scalar.activation(out=scols[:, 8 + tl:9 + tl],
                                             in_=mv[:, 1:2], func=AF.Sqrt,
                                             bias=epsk_t,
                                             scale=1.0 / (SCALE * SCALE))
                        nc.vector.reciprocal(out=srstd_cols_k[:, it:it + 1],
                                             in_=scols[:, 8 + tl:9 + tl])
                return scols

            def stats_finish(nm, ic, scols):
                """PE transpose + row assembly; emitted just before the consumer."""
                ncol = 12 if nm == "t" else 8
                st_ps = stats_ps.tile([12, P], F32, tag="stats",
                                      name=f"stps_{nm}_{ic}")
                nc.tensor.transpose(st_ps[ds(0, ncol), :], in_=scols[:, 0:ncol],
                                    identity=ident)
                st_sb = pp.tile([12, P], BF16, tag="st_sb", bufs=2,
                                name=f"stsb_{nm}_{ic}")
                nc.vector.tensor_copy(out=st_sb[ds(0, ncol), :],
                                      in_=st_ps[ds(0, ncol), :])
                nc.sync.dma_start(out=corr_rows[nm][ds(0, 1), ts(ic, 512)],
                                  in_=st_sb[ds(0, 4), :])
                nc.sync.dma_start(out=corr_rows[nm][ds(1, 1), ts(ic, 512)],
                                  in_=st_sb[ds(4, 4), :])
                if nm == "t":
                    nc.sync.dma_start(out=rstd_dram[nm][ts(ic, 512)],
                                      in_=st_sb[ds(8, 4), :])
                    rd = rstd_dram[nm][ts(ic, 512)]
                    nc.sync.dma_start(
                        out=rstd_bc[nm][:, ts(ic, 512)],
                        in_=bass.AP(tensor=rd.tensor, offset=rd.offset,
                                    ap=[[0, P]] + list(rd.ap)))

            def stats_chunk(nm, ic):
                stats_finish(nm, ic, stats_dve(nm, ic))

            def load_w(nm):
                w_all = pp.tile([P, N_DB, D], BF16, tag="w_all", bufs=2,
                                name=f"w_all_{nm}")
                # two DMAs so the first e-half is usable before the second lands
                for h in range(2):
                    nc.sync.dma_start(
                        out=w_all[:, :, ts(h, 512)],
                        in_=w_d[nm][:, ts(h, 512)].rearrange("(a p) e -> p a e", p=P))
                return w_all

            def load_xt(nm, ic):
                xt_sb = pp.tile([P, N_DB, 512], BF16, tag="xt", bufs=3,
                                name=f"xt_{nm}_{ic}")
                nc.sync.dma_start(
                    out=xt_sb,
                    in_=x_tr[nm][:, ts(ic, 512)].rearrange("(a p) t -> p a t", p=P))
                return xt_sb

            def proj_qk(nm, dstT, w_all, xt_sb, ic):
                """dstT[:, eb, chunk ic] in fp8, LN+bias folded."""
                for eb in range(N_EB):
                    ps = mm_ps.tile([P, 512], F32, tag="mm",
                                    name=f"ps_{nm}_{ic}_{eb}")
                    for db in range(N_DB):
                        nc.tensor.matmul(ps, lhsT=w_all[:, db, ts(eb, P)],
                                         rhs=xt_sb[:, db, :],
                                         start=(db == 0), stop=False)
                    nc.tensor.matmul(ps, lhsT=corr_w[nm][:, ts(eb, P)],
                                     rhs=corr_rows[nm][:, ts(ic, 512)],
                                     start=False, stop=True)
                    if nm == "t":
                        nc.vector.tensor_mul(out=dstT[:, eb, ts(ic, 512)], in0=ps,
                                             in1=rstd_bc[nm][:, ts(ic, 512)])
                    else:
                        nc.scalar.activation(out=dstT[:, eb, ts(ic, 512)],
                                             in_=ps, func=AF.Identity)

            def proj_v(w_all, xt_sb, ic):
                for ec in range(N_EC):
                    for ml in range(4):
                        m = 4 * ic + ml
                        ps = mm_ps.tile([P, 512], F32, tag="mm",
                                        name=f"ps_v_{m}_{ec}")
                        for db in range(N_DB):
                            nc.tensor.matmul(ps,
                                             lhsT=xt_sb[:, db, ds(ml * P, P)],
                                             rhs=w_all[:, db, ts(ec, 512)],
                                             start=(db == 0), stop=False)
                        nc.tensor.matmul(ps, lhsT=corr_rows["v"][:, ts(m, P)],
                                         rhs=corr_w["v"][:, ts(ec, 512)],
                                         start=False, stop=True)
                        nc.scalar.activation(out=v_sb[:, m, ts(ec, 512)], in_=ps,
                                             func=AF.Identity,
                                             scale=rstd_cols_v[:, m:m + 1])

            # Each tensor's stats chain is emitted one projection-phase early
            # so the in-order DVE/PE queues have it ready when the projection
            # needs corr rows (avoids a stall at each phase transition).
            w_k = load_w("k")
            with nc.named_scope("proj_k"):
                scols_k = stats_dve("k", 0)
                for ic in range(N_IC):
                    xt_sb = load_xt("k", ic)
                    stats_finish("k", ic, scols_k)
                    if ic + 1 < N_IC:
                        scols_k = stats_dve("k", ic + 1)
                    proj_qk("k", kT, w_k, xt_sb, ic)
                    stats_chunk("t", ic)
            w_q = load_w("t")
            with nc.named_scope("proj_q"):
                for ic in range(N_IC):
                    xt_sb = load_xt("t", ic)
                    scols_v = stats_dve("v", ic)
                    proj_qk("t", qT, w_q, xt_sb, ic)
                    stats_finish("v", ic, scols_v)
            w_v = load_w("v")
            with nc.named_scope("proj_v"):
                for ic in range(N_IC):
                    xt_sb = load_xt("v", ic)
                    proj_v(w_v, xt_sb, ic)

        # ---- attention ----
        with tc.tile_pool(name="attv_ps", bufs=2, space="PSUM") as attv_ps, \
             tc.tile_pool(name="sc_ps", bufs=2, space="PSUM") as sc_ps, \
             tc.tile_pool(name="att", bufs=1) as att:
            for ic in range(N_IC):
                with nc.named_scope(f"scores_{ic}"):
                    aT = att.tile([P, N_IT, 512], BF16, tag="aT", bufs=2,
                                  name=f"aT_{ic}")
                    for jt in range(N_IT):
                        ps = sc_ps.tile([P, 512], F32, tag="sc",
                                        name=f"ps_s_{ic}_{jt}")
                        for ebp in range(N_EB // 2):
                            nc.tensor.matmul(
                                ps, lhsT=kT[:, ds(2 * ebp, 2), ts(jt, P)],
                                rhs=qT[:, ds(2 * ebp, 2), ts(ic, 512)],
                                start=(ebp == 0), stop=(ebp == N_EB // 2 - 1),
                                perf_mode=mybir.MatmulPerfMode.DoubleRow)
                        nc.scalar.activation(
                            out=aT[:, jt, :], in_=ps,
                            func=AF.Exp, scale=srstd_cols_k[:, jt:jt + 1])
                with nc.named_scope(f"attv_{ic}"):
                    for isub in range(4):
                        ou = attv_ps.tile([P, D], F32, tag="ou",
                                          name=f"ou_{ic}_{isub}")
                        zz = sc_ps.tile([P, 1], F32, tag="z",
                                        name=f"z_{ic}_{isub}")
                        # same-bank runs of 16 accumulating matmuls (bank cycling
                        # between consecutive matmuls forces PE micro-stalls)
                        for ec in range(N_EC):
                            for jt in range(N_IT):
                                nc.tensor.matmul(
                                    ou[:, ts(ec, 512)],
                                    lhsT=aT[:, jt, ts(isub, P)],
                                    rhs=v_sb[:, jt, ts(ec, 512)],
                                    start=(jt == 0), stop=(jt == N_IT - 1))
                        for jt in range(N_IT):
                            nc.tensor.matmul(zz, lhsT=aT[:, jt, ts(isub, P)],
                                             rhs=ones_t,
                                             start=(jt == 0), stop=(jt == N_IT - 1))
                        rz = att.tile([P, 1], F32, tag="rz", bufs=2,
                                      name=f"rz_{ic}_{isub}")
                        nc.vector.reciprocal(out=rz, in_=zz)
                        o_sb = att.tile([P, D], F32, tag="o_sb", bufs=2,
                                        name=f"o_{ic}_{isub}")
                        nc.vector.tensor_scalar_mul(out=o_sb, in0=ou, scalar1=rz)
                        nc.sync.dma_start(out=out[ts(ic * 4 + isub, P), :], in_=o_sb)

    nc.compile()
    return nc


_NC_CACHE = None


def _get_module():
    global _NC_CACHE
    if _NC_CACHE is None:
        _NC_CACHE = build_module()
    return _NC_CACHE


def host_prep(target, source_k, source_v, Wq, bq, Wk, bk, Wv, bv,
              g_t, b_t, g_k, b_k, g_v, b_v):
    """Shared host-side input prep; returns per-core in_maps."""
    bf16 = ml_dtypes.bfloat16
    f32 = np.float32
    Wq = np.asarray(Wq, f32); bq = np.asarray(bq, f32)
    Wk = np.asarray(Wk, f32); bk = np.asarray(bk, f32)
    Wv = np.asarray(Wv, f32); bv = np.asarray(bv, f32)
    g_t = np.asarray(g_t, f32); b_t = np.asarray(b_t, f32)
    g_k = np.asarray(g_k, f32); b_k = np.asarray(b_k, f32)
    g_v = np.asarray(g_v, f32); b_v = np.asarray(b_v, f32)

    # Fold the layernorm affine (g, b) into the projection weights/biases:
    #   LN_affine(x) @ W.T + b  ==  LN_plain(x) @ (W*g).T + (b + W @ b_ln)
    wts = {"t": np.ascontiguousarray((Wq * g_t[None, :]).T).astype(bf16),
           "k": np.ascontiguousarray((Wk * g_k[None, :]).T).astype(bf16),
           "v": np.ascontiguousarray((Wv * g_v[None, :]).T).astype(bf16)}
    bias = {"t": (bq + Wq @ b_t).astype(bf16), "k": (bk + Wk @ b_k).astype(bf16),
            "v": (bv + Wv @ b_v).astype(bf16)}
    csum = {nm: wts[nm].astype(f32).sum(axis=0).astype(bf16) for nm in wts}
    ident = np.eye(P, dtype=f32)

    xs = {"t": np.asarray(target, f32), "k": np.asarray(source_k, f32),
          "v": np.asarray(source_v, f32)}
    in_maps = []
    for b in range(B):
        im = {"ident": ident}
        for nm in ("t", "k", "v"):
            im[f"x_{nm}"] = np.ascontiguousarray(xs[nm][b]).astype(bf16)
            im[f"xt_{nm}"] = np.ascontiguousarray(xs[nm][b].T).astype(bf16)
            im[f"w_{nm}"] = wts[nm]
            im[f"cs_{nm}"] = csum[nm]
            im[f"b_{nm}"] = bias[nm]
        in_maps.append(im)
    return in_maps


def kernel(target, source_k, source_v, Wq, bq, Wk, bk, Wv, bv,
           g_t, b_t, g_k, b_k, g_v, b_v):
    in_maps = host_prep(target, source_k, source_v, Wq, bq, Wk, bk, Wv, bv,
                        g_t, b_t, g_k, b_k, g_v, b_v)
    nc = _get_module()
    res = run_bass_kernel_spmd(nc, in_maps, core_ids=list(range(B)),
                               trace=bool(int(os.environ.get("KERNEL_TRACE", "0"))))
    out = np.stack([res.results[b]["out"] for b in range(B)], axis=0)
    kernel.last_results = res
    return out


# revision 7
# speedup vs baseline: 1.0406x; 1.0052x over previous
"""Single-head cross-attention (layernorm + QKV proj + softmax(QK^T)V) on 8 NeuronCores.

Sharding: data-parallel over batch B=8, one batch element per core.

v2 design (vs the DMA-xbar-transpose baseline):
  * Host ships BOTH natural fp32 x (for stats) and pre-transposed bf16 x^T
    (for matmuls) — no on-device activation transposes at all.
  * Layernorm is folded into the projections:
      q^T[e,i] = rstd_i * (W^T x^T_raw + colsum_w ⊗ (-mu) + b ⊗ inv_rstd)[e,i]
    The two rank-1 corrections ride a single K=2 matmul appended to each
    PSUM accumulation group; the per-token rstd scale is applied at PSUM
    evacuation (DVE tensor_mul with a broadcast rstd row for q/k in
    transposed layout; ScalarE per-partition scale for v in natural layout).
  * Stats rows (free-dim layout) are produced by a tiny PE transpose of the
    per-chunk stats columns; rstd broadcast goes through a DRAM round-trip.
  * Scores K^T·Q run in fp8(e4m3) with DoubleRow perf mode (2 d-blocks per
    matmul) — rel err ~1.2e-2 (gate 2e-2), everything else bf16/fp32.
  * attn·V and the Z (softmax denominator) matmuls as in the baseline.
"""

import os
from contextlib import ExitStack

import numpy as np
import ml_dtypes

import concourse.bass as bass
import concourse.bacc as bacc
import concourse.mybir as mybir
import concourse.tile as tile
from concourse.bass import ts, ds
from concourse.bass_utils import run_bass_kernel_spmd

BF16 = mybir.dt.bfloat16
F32 = mybir.dt.float32
F8 = mybir.dt.float8e4

B, T, D = 8, 2048, 1024
EPS = 1e-5
SCALE = float(D) ** -0.5
P = 128
N_IT = T // P          # 16 token tiles of 128
N_DB = D // P          # 8 d-blocks of 128
N_EB = D // P          # 8 e-blocks of 128
N_IC = T // 512        # 4 token chunks of 512
N_EC = D // 512        # 2 e chunks of 512

AF = mybir.ActivationFunctionType


def build_module() -> bass.Bass:
    nc = bacc.Bacc("TRN2", target_bir_lowering=False)

    x_nat = {}
    x_tr = {}
    w_d = {}
    cs_d = {}
    b_d = {}
    for nm in ("t", "k", "v"):
        x_nat[nm] = nc.dram_tensor(f"x_{nm}", [T, D], BF16, kind="ExternalInput")
        # pre-shuffled on host to match SBUF tile layouts (big DMA runs):
        # xt[ic, p, db, t_local] = x[ic*512+t, db*128+p];  w{lo,hi}[p, db, e]
        x_tr[nm] = nc.dram_tensor(f"xt_{nm}", [N_IC, P, N_DB, 512], BF16,
                                  kind="ExternalInput")
        w_d[nm] = [nc.dram_tensor(f"w_{nm}{h}", [P, N_DB, 512], BF16,
                                  kind="ExternalInput") for h in range(2)]
        cs_d[nm] = nc.dram_tensor(f"cs_{nm}", [D], BF16, kind="ExternalInput")
        b_d[nm] = nc.dram_tensor(f"b_{nm}", [D], BF16, kind="ExternalInput")
    ident_d = nc.dram_tensor("ident", [P, P], F32, kind="ExternalInput")
    out = nc.dram_tensor("out", [T, D], F32, kind="ExternalOutput")
    rstd_dram = {nm: nc.dram_tensor(f"rstd_d_{nm}", [T], BF16, kind="Internal")
                 for nm in ("t",)}

    with tile.TileContext(nc) as tc, ExitStack() as ctx:
        const = ctx.enter_context(tc.tile_pool(name="const", bufs=1))
        qkv = ctx.enter_context(tc.tile_pool(name="qkv", bufs=1))

        # ---- constants ----
        eps_t = const.tile([P, 1], F32)
        nc.vector.memset(eps_t, EPS)
        epsk_t = const.tile([P, 1], F32)
        nc.vector.memset(epsk_t, EPS / (SCALE * SCALE))
        ones_t = const.tile([P, 1], BF16)
        nc.vector.memset(ones_t, 1.0)
        ident = const.tile([P, P], F32)
        nc.sync.dma_start(out=ident, in_=ident_d[:, :])

        # corr_w[X]: partition0 = colsum(w_eff), partition1 = folded bias.
        # bf16: fp32 matmuls run at 1/4 rate (2 half-speed passes) and the
        # correction terms are small relative to q/k/v (~3% magnitude).
        corr_w = {}
        for nm in ("t", "k", "v"):
            cw = qkv.tile([2, D], BF16, name=f"corr_w_{nm}")
            nc.sync.dma_start(out=cw[ds(0, 1), :], in_=cs_d[nm][:].unsqueeze(0))
            nc.sync.dma_start(out=cw[ds(1, 1), :], in_=b_d[nm][:].unsqueeze(0))
            corr_w[nm] = cw

        # ---- persistent projection outputs ----
        qT = qkv.tile([P, N_EB, T], F8)          # qT[p, eb, i] = q[i, eb*128+p]
        kT = qkv.tile([P, N_EB, T], F8)
        v_sb = qkv.tile([P, N_IT, D], BF16)      # v[p, jt, e] = v[jt*128+p, e]
        corr_rows = {nm: qkv.tile([2, T], BF16, name=f"corr_rows_{nm}")
                     for nm in ("t", "k", "v")}  # p0 = -mu, p1 = sqrt(var+eps)
        rstd_bc = {nm: qkv.tile([P, T], BF16, name=f"rstd_bc_{nm}")
                   for nm in ("t",)}             # rstd broadcast along partitions
        rstd_cols_v = qkv.tile([P, N_IT], F32)   # v rstd, natural col layout
        srstd_cols_k = qkv.tile([P, N_IT], F32)  # SCALE * rstd_k, folded into exp

        with tc.tile_pool(name="stats_ps", bufs=2, space="PSUM") as stats_ps, \
             tc.tile_pool(name="mm_ps", bufs=6, space="PSUM") as mm_ps, \
             tc.tile_pool(name="proj_phase", bufs=1) as pp:

            def stats_dve(nm, ic):
                """DVE/ACT part of the stats chain for one token chunk."""
                scols = pp.tile([P, 12], F32, tag="scols", bufs=3,
                                name=f"scols_{nm}_{ic}")
                for tl in range(4):
                    it = 4 * ic + tl
                    x_raw = pp.tile([P, D], BF16, tag="x_raw", bufs=4,
                                    name=f"x_raw_{nm}_{it}")
                    nc.gpsimd.dma_start(out=x_raw, in_=x_nat[nm][ts(it, P), :])
                    st6 = pp.tile([P, 2, 6], F32, tag="st6", bufs=3,
                                  name=f"st6_{nm}_{it}")
                    for sb in range(2):
                        nc.vector.bn_stats(out=st6[:, sb, :], in_=x_raw[:, ts(sb, 512)])
                    mv = pp.tile([P, 2], F32, tag="mv", bufs=3,
                                 name=f"mv_{nm}_{it}")
                    nc.vector.bn_aggr(out=mv, in_=st6)
                    # col 0..3: -mu ; col 4..7: sqrt(var+eps) ; col 8..11: rstd
                    nc.scalar.activation(out=scols[:, tl:tl + 1], in_=mv[:, 0:1],
                                         func=AF.Copy, scale=-1.0)
                    nc.scalar.activation(out=scols[:, 4 + tl:5 + tl], in_=mv[:, 1:2],
                                         func=AF.Sqrt, bias=eps_t)
                    if nm == "t":
                        # rstd row -> DRAM round-trip -> partition broadcast
                        nc.vector.reciprocal(out=scols[:, 8 + tl:9 + tl],
                                             in_=scols[:, 4 + tl:5 + tl])
                    elif nm == "v":
                        nc.vector.reciprocal(out=rstd_cols_v[:, it:it + 1],
                                             in_=scols[:, 4 + tl:5 + tl])
                    else:
                        # k: exp folds SCALE*rstd_j per partition; build
                        # 1/(SCALE^-1 * sqrt(var+eps)) via a scaled sqrt
                        nc.scalar.activation(out=scols[:, 8 + tl:9 + tl],
                                             in_=mv[:, 1:2], func=AF.Sqrt,
                                             bias=epsk_t,
                                             scale=1.0 / (SCALE * SCALE))
                        nc.vector.reciprocal(out=srstd_cols_k[:, it:it + 1],
                                             in_=scols[:, 8 + tl:9 + tl])
                return scols

            def stats_finish(nm, ic, scols):
                """PE transpose + row assembly; emitted just before the consumer."""
                ncol = 12 if nm == "t" else 8
                st_ps = stats_ps.tile([12, P], F32, tag="stats",
                                      name=f"stps_{nm}_{ic}")
                nc.tensor.transpose(st_ps[ds(0, ncol), :], in_=scols[:, 0:ncol],
                                    identity=ident)
                st_sb = pp.tile([12, P], BF16, tag="st_sb", bufs=2,
                                name=f"stsb_{nm}_{ic}")
                nc.vector.tensor_copy(out=st_sb[ds(0, ncol), :],
                                      in_=st_ps[ds(0, ncol), :])
                nc.sync.dma_start(out=corr_rows[nm][ds(0, 1), ts(ic, 512)],
                                  in_=st_sb[ds(0, 4), :])
                nc.sync.dma_start(out=corr_rows[nm][ds(1, 1), ts(ic, 512)],
                                  in_=st_sb[ds(4, 4), :])
                if nm == "t":
                    nc.sync.dma_start(out=rstd_dram[nm][ts(ic, 512)],
                                      in_=st_sb[ds(8, 4), :])
                    rd = rstd_dram[nm][ts(ic, 512)]
                    nc.sync.dma_start(
                        out=rstd_bc[nm][:, ts(ic, 512)],
                        in_=bass.AP(tensor=rd.tensor, offset=rd.offset,
                                    ap=[[0, P]] + list(rd.ap)))

            def stats_chunk(nm, ic):
                stats_finish(nm, ic, stats_dve(nm, ic))

            def load_w(nm):
                halves = []
                for h in range(2):
                    wh = pp.tile([P, N_DB, 512], BF16, tag="w_half", bufs=4,
                                 name=f"w_{nm}_{h}")
                    nc.sync.dma_start(out=wh, in_=w_d[nm][h][:, :, :])
                    halves.append(wh)
                return halves

            def load_xt(nm, ic):
                xt_sb = pp.tile([P, N_DB, 512], BF16, tag="xt", bufs=3,
                                name=f"xt_{nm}_{ic}")
                nc.sync.dma_start(out=xt_sb, in_=x_tr[nm][ic, :, :, :])
                return xt_sb

            def proj_qk(nm, dstT, w_all, xt_sb, ic):
                """dstT[:, eb, chunk ic] in fp8, LN+bias folded."""
                for eb in range(N_EB):
                    ps = mm_ps.tile([P, 512], F32, tag="mm",
                                    name=f"ps_{nm}_{ic}_{eb}")
                    wh = w_all[eb // 4][:, :, ts(eb % 4, P)]
                    for db in range(N_DB):
                        nc.tensor.matmul(ps, lhsT=wh[:, db, :],
                                         rhs=xt_sb[:, db, :],
                                         start=(db == 0), stop=False)
                    nc.tensor.matmul(ps, lhsT=corr_w[nm][:, ts(eb, P)],
                                     rhs=corr_rows[nm][:, ts(ic, 512)],
                                     start=False, stop=True)
                    if nm == "t":
                        nc.vector.tensor_mul(out=dstT[:, eb, ts(ic, 512)], in0=ps,
                                             in1=rstd_bc[nm][:, ts(ic, 512)])
                    else:
                        nc.scalar.activation(out=dstT[:, eb, ts(ic, 512)],
                                             in_=ps, func=AF.Identity)

            def proj_v(w_all, xt_sb, ic):
                for ec in range(N_EC):
                    for ml in range(4):
                        m = 4 * ic + ml
                        ps = mm_ps.tile([P, 512], F32, tag="mm",
                                        name=f"ps_v_{m}_{ec}")
                        for db in range(N_DB):
                            nc.tensor.matmul(ps,
                                             lhsT=xt_sb[:, db, ds(ml * P, P)],
                                             rhs=w_all[ec][:, db, :],
                                             start=(db == 0), stop=False)
                        nc.tensor.matmul(ps, lhsT=corr_rows["v"][:, ts(m, P)],
                                         rhs=corr_w["v"][:, ts(ec, 512)],
                                         start=False, stop=True)
                        nc.scalar.activation(out=v_sb[:, m, ts(ec, 512)], in_=ps,
                                             func=AF.Identity,
                                             scale=rstd_cols_v[:, m:m + 1])

            # Each tensor's stats chain is emitted one projection-phase early
            # so the in-order DVE/PE queues have it ready when the projection
            # needs corr rows (avoids a stall at each phase transition).
            w_k = load_w("k")
            with nc.named_scope("proj_k"):
                scols_k = stats_dve("k", 0)
                for ic in range(N_IC):
                    xt_sb = load_xt("k", ic)
                    stats_finish("k", ic, scols_k)
                    if ic + 1 < N_IC:
                        scols_k = stats_dve("k", ic + 1)
                    proj_qk("k", kT, w_k, xt_sb, ic)
                    stats_chunk("t", ic)
            w_q = load_w("t")
            with nc.named_scope("proj_q"):
                for ic in range(N_IC):
                    xt_sb = load_xt("t", ic)
                    scols_v = stats_dve("v", ic)
                    proj_qk("t", qT, w_q, xt_sb, ic)
                    stats_finish("v", ic, scols_v)
            w_v = load_w("v")
            with nc.named_scope("proj_v"):
                for ic in range(N_IC):
                    xt_sb = load_xt("v", ic)
                    proj_v(w_v, xt_sb, ic)

        # ---- attention ----
        with tc.tile_pool(name="attv_ps", bufs=2, space="PSUM") as attv_ps, \
             tc.tile_pool(name="sc_ps", bufs=2, space="PSUM") as sc_ps, \
             tc.tile_pool(name="att", bufs=1) as att:
            for ic in range(N_IC):
                with nc.named_scope(f"scores_{ic}"):
                    aT = att.tile([P, N_IT, 512], BF16, tag="aT", bufs=2,
                                  name=f"aT_{ic}")
                    for jt in range(N_IT):
                        ps = sc_ps.tile([P, 512], F32, tag="sc",
                                        name=f"ps_s_{ic}_{jt}")
                        for ebp in range(N_EB // 2):
                            nc.tensor.matmul(
                                ps, lhsT=kT[:, ds(2 * ebp, 2), ts(jt, P)],
                                rhs=qT[:, ds(2 * ebp, 2), ts(ic, 512)],
                                start=(ebp == 0), stop=(ebp == N_EB // 2 - 1),
                                perf_mode=mybir.MatmulPerfMode.DoubleRow)
                        nc.scalar.activation(
                            out=aT[:, jt, :], in_=ps,
                            func=AF.Exp, scale=srstd_cols_k[:, jt:jt + 1])
                with nc.named_scope(f"attv_{ic}"):
                    for isub in range(4):
                        ou = attv_ps.tile([P, D], F32, tag="ou",
                                          name=f"ou_{ic}_{isub}")
                        zz = sc_ps.tile([P, 1], F32, tag="z",
                                        name=f"z_{ic}_{isub}")
                        # same-bank runs of 16 accumulating matmuls (bank cycling
                        # between consecutive matmuls forces PE micro-stalls);
                        # Z between the halves so rz is ready when ec0 evacuates
                        rz = att.tile([P, 1], F32, tag="rz", bufs=2,
                                      name=f"rz_{ic}_{isub}")
                        o_sb = att.tile([P, D], F32, tag="o_sb", bufs=2,
                                        name=f"o_{ic}_{isub}")
                        for jt in range(N_IT):
                            nc.tensor.matmul(
                                ou[:, ts(0, 512)], lhsT=aT[:, jt, ts(isub, P)],
                                rhs=v_sb[:, jt, ts(0, 512)],
                                start=(jt == 0), stop=(jt == N_IT - 1))
                        for jt in range(N_IT):
                            nc.tensor.matmul(zz, lhsT=aT[:, jt, ts(isub, P)],
                                             rhs=ones_t,
                                             start=(jt == 0), stop=(jt == N_IT - 1))
                        nc.vector.reciprocal(out=rz, in_=zz)
                        for jt in range(N_IT):
                            nc.tensor.matmul(
                                ou[:, ts(1, 512)], lhsT=aT[:, jt, ts(isub, P)],
                                rhs=v_sb[:, jt, ts(1, 512)],
                                start=(jt == 0), stop=(jt == N_IT - 1))
                        for ec in range(N_EC):
                            nc.vector.tensor_scalar_mul(out=o_sb[:, ts(ec, 512)],
                                                        in0=ou[:, ts(ec, 512)],
                                                        scalar1=rz)
                            nc.sync.dma_start(
                                out=out[ts(ic * 4 + isub, P), ts(ec, 512)],
                                in_=o_sb[:, ts(ec, 512)])

    nc.compile()
    return nc


_NC_CACHE = None


def _get_module():
    global _NC_CACHE
    if _NC_CACHE is None:
        _NC_CACHE = build_module()
    return _NC_CACHE


def host_prep(target, source_k, source_v, Wq, bq, Wk, bk, Wv, bv,
              g_t, b_t, g_k, b_k, g_v, b_v):
    """Shared host-side input prep; returns per-core in_maps."""
    bf16 = ml_dtypes.bfloat16
    f32 = np.float32
    Wq = np.asarray(Wq, f32); bq = np.asarray(bq, f32)
    Wk = np.asarray(Wk, f32); bk = np.asarray(bk, f32)
    Wv = np.asarray(Wv, f32); bv = np.asarray(bv, f32)
    g_t = np.asarray(g_t, f32); b_t = np.asarray(b_t, f32)
    g_k = np.asarray(g_k, f32); b_k = np.asarray(b_k, f32)
    g_v = np.asarray(g_v, f32); b_v = np.asarray(b_v, f32)

    # Fold the layernorm affine (g, b) into the projection weights/biases:
    #   LN_affine(x) @ W.T + b  ==  LN_plain(x) @ (W*g).T + (b + W @ b_ln)
    wts = {"t": np.ascontiguousarray((Wq * g_t[None, :]).T).astype(bf16),
           "k": np.ascontiguousarray((Wk * g_k[None, :]).T).astype(bf16),
           "v": np.ascontiguousarray((Wv * g_v[None, :]).T).astype(bf16)}
    # [d, e] -> [p, db, e] partition-shuffled, split into e-halves
    wsh = {nm: np.ascontiguousarray(
        wts[nm].reshape(N_DB, P, D).transpose(1, 0, 2)) for nm in wts}
    bias = {"t": (bq + Wq @ b_t).astype(bf16), "k": (bk + Wk @ b_k).astype(bf16),
            "v": (bv + Wv @ b_v).astype(bf16)}
    csum = {nm: wts[nm].astype(f32).sum(axis=0).astype(bf16) for nm in wts}
    ident = np.eye(P, dtype=f32)

    xs = {"t": np.asarray(target, f32), "k": np.asarray(source_k, f32),
          "v": np.asarray(source_v, f32)}
    in_maps = []
    for b in range(B):
        im = {"ident": ident}
        for nm in ("t", "k", "v"):
            im[f"x_{nm}"] = np.ascontiguousarray(xs[nm][b]).astype(bf16)
            xt2 = xs[nm][b].T.astype(bf16)      # [D, T]
            im[f"xt_{nm}"] = np.ascontiguousarray(
                xt2.reshape(N_DB, P, N_IC, 512).transpose(2, 1, 0, 3))
            im[f"w_{nm}0"] = np.ascontiguousarray(wsh[nm][:, :, :512])
            im[f"w_{nm}1"] = np.ascontiguousarray(wsh[nm][:, :, 512:])
            im[f"cs_{nm}"] = csum[nm]
            im[f"b_{nm}"] = bias[nm]
        in_maps.append(im)
    return in_maps


def kernel(target, source_k, source_v, Wq, bq, Wk, bk, Wv, bv,
           g_t, b_t, g_k, b_k, g_v, b_v):
    in_maps = host_prep(target, source_k, source_v, Wq, bq, Wk, bk, Wv, bv,
                        g_t, b_t, g_k, b_k, g_v, b_v)
    nc = _get_module()
    res = run_bass_kernel_spmd(nc, in_maps, core_ids=list(range(B)),
                               trace=bool(int(os.environ.get("KERNEL_TRACE", "0"))))
    out = np.stack([res.results[b]["out"] for b in range(B)], axis=0)
    kernel.last_results = res
    return out


# revision 8
# speedup vs baseline: 1.0897x; 1.0472x over previous
"""Single-head cross-attention (layernorm + QKV proj + softmax(QK^T)V) on 8 NeuronCores.

Sharding: data-parallel over batch B=8, one batch element per core.

v2 design (vs the DMA-xbar-transpose baseline):
  * Host ships BOTH natural fp32 x (for stats) and pre-transposed bf16 x^T
    (for matmuls) — no on-device activation transposes at all.
  * Layernorm is folded into the projections:
      q^T[e,i] = rstd_i * (W^T x^T_raw + colsum_w ⊗ (-mu) + b ⊗ inv_rstd)[e,i]
    The two rank-1 corrections ride a single K=2 matmul appended to each
    PSUM accumulation group; the per-token rstd scale is applied at PSUM
    evacuation (DVE tensor_mul with a broadcast rstd row for q/k in
    transposed layout; ScalarE per-partition scale for v in natural layout).
  * Stats rows (free-dim layout) are produced by a tiny PE transpose of the
    per-chunk stats columns; rstd broadcast goes through a DRAM round-trip.
  * Scores K^T·Q run in fp8(e4m3) with DoubleRow perf mode (2 d-blocks per
    matmul) — rel err ~1.2e-2 (gate 2e-2), everything else bf16/fp32.
  * attn·V and the Z (softmax denominator) matmuls as in the baseline.
"""

import os
from contextlib import ExitStack

import numpy as np
import ml_dtypes

import concourse.bass as bass
import concourse.bacc as bacc
import concourse.mybir as mybir
import concourse.tile as tile
from concourse.bass import ts, ds
from concourse.bass_utils import run_bass_kernel_spmd

BF16 = mybir.dt.bfloat16
F32 = mybir.dt.float32
F8 = mybir.dt.float8e4

B, T, D = 8, 2048, 1024
EPS = 1e-5
SCALE = float(D) ** -0.5
P = 128
N_IT = T // P          # 16 token tiles of 128
N_DB = D // P          # 8 d-blocks of 128
N_EB = D // P          # 8 e-blocks of 128
N_IC = T // 512        # 4 token chunks of 512
N_EC = D // 512        # 2 e chunks of 512

AF = mybir.ActivationFunctionType


def build_module() -> bass.Bass:
    nc = bacc.Bacc("TRN2", target_bir_lowering=False)

    x_nat = {}
    x_tr = {}
    w_d = {}
    cs_d = {}
    b_d = {}
    for nm in ("t", "k", "v"):
        x_nat[nm] = nc.dram_tensor(f"x_{nm}", [T, D], BF16, kind="ExternalInput")
        # pre-shuffled on host to match SBUF tile layouts (big DMA runs):
        # xt[ic, p, db, t_local] = x[ic*512+t, db*128+p];  w{lo,hi}[p, db, e]
        x_tr[nm] = nc.dram_tensor(f"xt_{nm}", [N_IC, P, N_DB, 512], BF16,
                                  kind="ExternalInput")
        w_d[nm] = [nc.dram_tensor(f"w_{nm}{h}", [P, N_DB, 512], BF16,
                                  kind="ExternalInput") for h in range(2)]
        cs_d[nm] = nc.dram_tensor(f"cs_{nm}", [D], BF16, kind="ExternalInput")
        b_d[nm] = nc.dram_tensor(f"b_{nm}", [D], BF16, kind="ExternalInput")
    ident_d = nc.dram_tensor("ident", [P, P], F32, kind="ExternalInput")
    out = nc.dram_tensor("out", [T, D], F32, kind="ExternalOutput")
    rstd_dram = {nm: nc.dram_tensor(f"rstd_d_{nm}", [T], BF16, kind="Internal")
                 for nm in ("t",)}

    with tile.TileContext(nc) as tc, ExitStack() as ctx:
        const = ctx.enter_context(tc.tile_pool(name="const", bufs=1))
        qkv = ctx.enter_context(tc.tile_pool(name="qkv", bufs=1))

        # ---- constants ----
        eps_t = const.tile([P, 1], F32)
        nc.vector.memset(eps_t, EPS)
        epsk_t = const.tile([P, 1], F32)
        nc.vector.memset(epsk_t, EPS / (SCALE * SCALE))
        ones_t = const.tile([P, 1], BF16)
        nc.vector.memset(ones_t, 1.0)
        ident = const.tile([P, P], F32)
        nc.sync.dma_start(out=ident, in_=ident_d[:, :])

        # corr_w[X]: partition0 = colsum(w_eff), partition1 = folded bias.
        # bf16: fp32 matmuls run at 1/4 rate (2 half-speed passes) and the
        # correction terms are small relative to q/k/v (~3% magnitude).
        # padded to K=128 with zero rows: a K=2 matmul measures ~85ns slower
        # than the uniform 128x128x512 shape and perturbs the PE pipeline
        corr_w = {}
        for nm in ("t", "k", "v"):
            cw = qkv.tile([P, D], BF16, name=f"corr_w_{nm}")
            nc.vector.memset(cw, 0.0)
            nc.sync.dma_start(out=cw[ds(0, 1), :], in_=cs_d[nm][:].unsqueeze(0))
            nc.sync.dma_start(out=cw[ds(1, 1), :], in_=b_d[nm][:].unsqueeze(0))
            corr_w[nm] = cw

        # ---- persistent projection outputs ----
        qT = qkv.tile([P, N_EB, T], F8)          # qT[p, eb, i] = q[i, eb*128+p]
        kT = qkv.tile([P, N_EB, T], F8)
        v_sb = qkv.tile([P, N_IT, D], BF16)      # v[p, jt, e] = v[jt*128+p, e]
        corr_rows = {nm: qkv.tile([P, T], BF16, name=f"corr_rows_{nm}")
                     for nm in ("t", "k", "v")}  # p0 = -mu, p1 = sqrt(var+eps)
        for nm in ("t", "k", "v"):
            nc.vector.memset(corr_rows[nm], 0.0)
        rstd_bc = {nm: qkv.tile([P, T], BF16, name=f"rstd_bc_{nm}")
                   for nm in ("t",)}             # rstd broadcast along partitions
        rstd_cols_v = qkv.tile([P, N_IT], F32)   # v rstd, natural col layout
        srstd_cols_k = qkv.tile([P, N_IT], F32)  # SCALE * rstd_k, folded into exp

        with tc.tile_pool(name="stats_ps", bufs=2, space="PSUM") as stats_ps, \
             tc.tile_pool(name="mm_ps", bufs=6, space="PSUM") as mm_ps, \
             tc.tile_pool(name="proj_phase", bufs=1) as pp:

            def stats_dve(nm, ic):
                """DVE/ACT part of the stats chain for one token chunk."""
                scols = pp.tile([P, 12], F32, tag="scols", bufs=3,
                                name=f"scols_{nm}_{ic}")
                for tl in range(4):
                    it = 4 * ic + tl
                    x_raw = pp.tile([P, D], BF16, tag="x_raw", bufs=4,
                                    name=f"x_raw_{nm}_{it}")
                    nc.gpsimd.dma_start(out=x_raw, in_=x_nat[nm][ts(it, P), :])
                    st6 = pp.tile([P, 2, 6], F32, tag="st6", bufs=3,
                                  name=f"st6_{nm}_{it}")
                    for sb in range(2):
                        nc.vector.bn_stats(out=st6[:, sb, :], in_=x_raw[:, ts(sb, 512)])
                    mv = pp.tile([P, 2], F32, tag="mv", bufs=3,
                                 name=f"mv_{nm}_{it}")
                    nc.vector.bn_aggr(out=mv, in_=st6)
                    # col 0..3: -mu ; col 4..7: sqrt(var+eps) ; col 8..11: rstd
                    nc.scalar.activation(out=scols[:, tl:tl + 1], in_=mv[:, 0:1],
                                         func=AF.Copy, scale=-1.0)
                    nc.scalar.activation(out=scols[:, 4 + tl:5 + tl], in_=mv[:, 1:2],
                                         func=AF.Sqrt, bias=eps_t)
                    if nm == "t":
                        # rstd row -> DRAM round-trip -> partition broadcast
                        nc.vector.reciprocal(out=scols[:, 8 + tl:9 + tl],
                                             in_=scols[:, 4 + tl:5 + tl])
                    elif nm == "v":
                        nc.vector.reciprocal(out=rstd_cols_v[:, it:it + 1],
                                             in_=scols[:, 4 + tl:5 + tl])
                    else:
                        # k: exp folds SCALE*rstd_j per partition; build
                        # 1/(SCALE^-1 * sqrt(var+eps)) via a scaled sqrt
                        nc.scalar.activation(out=scols[:, 8 + tl:9 + tl],
                                             in_=mv[:, 1:2], func=AF.Sqrt,
                                             bias=epsk_t,
                                             scale=1.0 / (SCALE * SCALE))
                        nc.vector.reciprocal(out=srstd_cols_k[:, it:it + 1],
                                             in_=scols[:, 8 + tl:9 + tl])
                return scols

            def stats_finish(nm, ic, scols):
                """PE transpose + row assembly; emitted just before the consumer."""
                ncol = 12 if nm == "t" else 8
                st_ps = stats_ps.tile([12, P], F32, tag="stats",
                                      name=f"stps_{nm}_{ic}")
                nc.tensor.transpose(st_ps[ds(0, ncol), :], in_=scols[:, 0:ncol],
                                    identity=ident)
                st_sb = pp.tile([12, P], BF16, tag="st_sb", bufs=2,
                                name=f"stsb_{nm}_{ic}")
                nc.vector.tensor_copy(out=st_sb[ds(0, ncol), :],
                                      in_=st_ps[ds(0, ncol), :])
                nc.sync.dma_start(out=corr_rows[nm][ds(0, 1), ts(ic, 512)],
                                  in_=st_sb[ds(0, 4), :])
                nc.sync.dma_start(out=corr_rows[nm][ds(1, 1), ts(ic, 512)],
                                  in_=st_sb[ds(4, 4), :])
                if nm == "t":
                    nc.sync.dma_start(out=rstd_dram[nm][ts(ic, 512)],
                                      in_=st_sb[ds(8, 4), :])
                    rd = rstd_dram[nm][ts(ic, 512)]
                    nc.sync.dma_start(
                        out=rstd_bc[nm][:, ts(ic, 512)],
                        in_=bass.AP(tensor=rd.tensor, offset=rd.offset,
                                    ap=[[0, P]] + list(rd.ap)))

            def stats_chunk(nm, ic):
                stats_finish(nm, ic, stats_dve(nm, ic))

            def load_w(nm):
                halves = []
                for h in range(2):
                    wh = pp.tile([P, N_DB, 512], BF16, tag="w_half", bufs=4,
                                 name=f"w_{nm}_{h}")
                    nc.sync.dma_start(out=wh, in_=w_d[nm][h][:, :, :])
                    halves.append(wh)
                return halves

            def load_xt(nm, ic):
                xt_sb = pp.tile([P, N_DB, 512], BF16, tag="xt", bufs=3,
                                name=f"xt_{nm}_{ic}")
                nc.sync.dma_start(out=xt_sb, in_=x_tr[nm][ic, :, :, :])
                return xt_sb

            def proj_qk(nm, dstT, w_all, xt_sb, ic):
                """dstT[:, eb, chunk ic] in fp8, LN+bias folded."""
                for eb in range(N_EB):
                    ps = mm_ps.tile([P, 512], F32, tag="mm",
                                    name=f"ps_{nm}_{ic}_{eb}")
                    wh = w_all[eb // 4][:, :, ts(eb % 4, P)]
                    for db in range(N_DB):
                        nc.tensor.matmul(ps, lhsT=wh[:, db, :],
                                         rhs=xt_sb[:, db, :],
                                         start=(db == 0), stop=False)
                    nc.tensor.matmul(ps, lhsT=corr_w[nm][:, ts(eb, P)],
                                     rhs=corr_rows[nm][:, ts(ic, 512)],
                                     start=False, stop=True)
                    if nm == "t":
                        nc.vector.tensor_mul(out=dstT[:, eb, ts(ic, 512)], in0=ps,
                                             in1=rstd_bc[nm][:, ts(ic, 512)])
                    else:
                        nc.scalar.activation(out=dstT[:, eb, ts(ic, 512)],
                                             in_=ps, func=AF.Identity)

            def proj_v(w_all, xt_sb, ic):
                for ec in range(N_EC):
                    for ml in range(4):
                        m = 4 * ic + ml
                        ps = mm_ps.tile([P, 512], F32, tag="mm",
                                        name=f"ps_v_{m}_{ec}")
                        for db in range(N_DB):
                            nc.tensor.matmul(ps,
                                             lhsT=xt_sb[:, db, ds(ml * P, P)],
                                             rhs=w_all[ec][:, db, :],
                                             start=(db == 0), stop=False)
                        nc.tensor.matmul(ps, lhsT=corr_rows["v"][:, ts(m, P)],
                                         rhs=corr_w["v"][:, ts(ec, 512)],
                                         start=False, stop=True)
                        nc.scalar.activation(out=v_sb[:, m, ts(ec, 512)], in_=ps,
                                             func=AF.Identity,
                                             scale=rstd_cols_v[:, m:m + 1])

            # Each tensor's stats chain is emitted one projection-phase early
            # so the in-order DVE/PE queues have it ready when the projection
            # needs corr rows (avoids a stall at each phase transition).
            w_k = load_w("k")
            with nc.named_scope("proj_k"):
                scols_k = stats_dve("k", 0)
                for ic in range(N_IC):
                    xt_sb = load_xt("k", ic)
                    stats_finish("k", ic, scols_k)
                    if ic + 1 < N_IC:
                        scols_k = stats_dve("k", ic + 1)
                    proj_qk("k", kT, w_k, xt_sb, ic)
                    stats_chunk("t", ic)
            w_q = load_w("t")
            with nc.named_scope("proj_q"):
                for ic in range(N_IC):
                    xt_sb = load_xt("t", ic)
                    scols_v = stats_dve("v", ic)
                    proj_qk("t", qT, w_q, xt_sb, ic)
                    stats_finish("v", ic, scols_v)
            w_v = load_w("v")
            with nc.named_scope("proj_v"):
                for ic in range(N_IC):
                    xt_sb = load_xt("v", ic)
                    proj_v(w_v, xt_sb, ic)

        # ---- attention ----
        with tc.tile_pool(name="attv_ps", bufs=2, space="PSUM") as attv_ps, \
             tc.tile_pool(name="sc_ps", bufs=2, space="PSUM") as sc_ps, \
             tc.tile_pool(name="att", bufs=1) as att:
            for ic in range(N_IC):
                with nc.named_scope(f"scores_{ic}"):
                    aT = att.tile([P, N_IT, 512], BF16, tag="aT", bufs=2,
                                  name=f"aT_{ic}")
                    for jt in range(N_IT):
                        ps = sc_ps.tile([P, 512], F32, tag="sc",
                                        name=f"ps_s_{ic}_{jt}")
                        for ebp in range(N_EB // 2):
                            nc.tensor.matmul(
                                ps, lhsT=kT[:, ds(2 * ebp, 2), ts(jt, P)],
                                rhs=qT[:, ds(2 * ebp, 2), ts(ic, 512)],
                                start=(ebp == 0), stop=(ebp == N_EB // 2 - 1),
                                perf_mode=mybir.MatmulPerfMode.DoubleRow)
                        nc.scalar.activation(
                            out=aT[:, jt, :], in_=ps,
                            func=AF.Exp, scale=srstd_cols_k[:, jt:jt + 1])
                with nc.named_scope(f"attv_{ic}"):
                    for isub in range(4):
                        ou = attv_ps.tile([P, D], F32, tag="ou",
                                          name=f"ou_{ic}_{isub}")
                        zz = sc_ps.tile([P, 1], F32, tag="z",
                                        name=f"z_{ic}_{isub}")
                        # same-bank runs of 16 accumulating matmuls (bank cycling
                        # between consecutive matmuls forces PE micro-stalls);
                        # Z between the halves so rz is ready when ec0 evacuates
                        rz = att.tile([P, 1], F32, tag="rz", bufs=2,
                                      name=f"rz_{ic}_{isub}")
                        o_sb = att.tile([P, D], F32, tag="o_sb", bufs=2,
                                        name=f"o_{ic}_{isub}")
                        for jt in range(N_IT):
                            nc.tensor.matmul(
                                ou[:, ts(0, 512)], lhsT=aT[:, jt, ts(isub, P)],
                                rhs=v_sb[:, jt, ts(0, 512)],
                                start=(jt == 0), stop=(jt == N_IT - 1))
                        for jt in range(N_IT):
                            nc.tensor.matmul(zz, lhsT=aT[:, jt, ts(isub, P)],
                                             rhs=ones_t,
                                             start=(jt == 0), stop=(jt == N_IT - 1))
                        nc.vector.reciprocal(out=rz, in_=zz)
                        for jt in range(N_IT):
                            nc.tensor.matmul(
                                ou[:, ts(1, 512)], lhsT=aT[:, jt, ts(isub, P)],
                                rhs=v_sb[:, jt, ts(1, 512)],
                                start=(jt == 0), stop=(jt == N_IT - 1))
                        for ec in range(N_EC):
                            nc.vector.tensor_scalar_mul(out=o_sb[:, ts(ec, 512)],
                                                        in0=ou[:, ts(ec, 512)],
                                                        scalar1=rz)
                            nc.sync.dma_start(
                                out=out[ts(ic * 4 + isub, P), ts(ec, 512)],
                                in_=o_sb[:, ts(ec, 512)])

    nc.compile()
    return nc


_NC_CACHE = None


def _get_module():
    global _NC_CACHE
    if _NC_CACHE is None:
        _NC_CACHE = build_module()
    return _NC_CACHE


def host_prep(target, source_k, source_v, Wq, bq, Wk, bk, Wv, bv,
              g_t, b_t, g_k, b_k, g_v, b_v):
    """Shared host-side input prep; returns per-core in_maps."""
    bf16 = ml_dtypes.bfloat16
    f32 = np.float32
    Wq = np.asarray(Wq, f32); bq = np.asarray(bq, f32)
    Wk = np.asarray(Wk, f32); bk = np.asarray(bk, f32)
    Wv = np.asarray(Wv, f32); bv = np.asarray(bv, f32)
    g_t = np.asarray(g_t, f32); b_t = np.asarray(b_t, f32)
    g_k = np.asarray(g_k, f32); b_k = np.asarray(b_k, f32)
    g_v = np.asarray(g_v, f32); b_v = np.asarray(b_v, f32)

    # Fold the layernorm affine (g, b) into the projection weights/biases:
    #   LN_affine(x) @ W.T + b  ==  LN_plain(x) @ (W*g).T + (b + W @ b_ln)
    wts = {"t": np.ascontiguousarray((Wq * g_t[None, :]).T).astype(bf16),
           "k": np.ascontiguousarray((Wk * g_k[None, :]).T).astype(bf16),
           "v": np.ascontiguousarray((Wv * g_v[None, :]).T).astype(bf16)}
    # [d, e] -> [p, db, e] partition-shuffled, split into e-halves
    wsh = {nm: np.ascontiguousarray(
        wts[nm].reshape(N_DB, P, D).transpose(1, 0, 2)) for nm in wts}
    bias = {"t": (bq + Wq @ b_t).astype(bf16), "k": (bk + Wk @ b_k).astype(bf16),
            "v": (bv + Wv @ b_v).astype(bf16)}
    csum = {nm: wts[nm].astype(f32).sum(axis=0).astype(bf16) for nm in wts}
    ident = np.eye(P, dtype=f32)

    xs = {"t": np.asarray(target, f32), "k": np.asarray(source_k, f32),
          "v": np.asarray(source_v, f32)}
    in_maps = []
    for b in range(B):
        im = {"ident": ident}
        for nm in ("t", "k", "v"):
            im[f"x_{nm}"] = np.ascontiguousarray(xs[nm][b]).astype(bf16)
            xt2 = xs[nm][b].T.astype(bf16)      # [D, T]
            im[f"xt_{nm}"] = np.ascontiguousarray(
                xt2.reshape(N_DB, P, N_IC, 512).transpose(2, 1, 0, 3))
            im[f"w_{nm}0"] = np.ascontiguousarray(wsh[nm][:, :, :512])
            im[f"w_{nm}1"] = np.ascontiguousarray(wsh[nm][:, :, 512:])
            im[f"cs_{nm}"] = csum[nm]
            im[f"b_{nm}"] = bias[nm]
        in_maps.append(im)
    return in_maps


def kernel(target, source_k, source_v, Wq, bq, Wk, bk, Wv, bv,
           g_t, b_t, g_k, b_k, g_v, b_v):
    in_maps = host_prep(target, source_k, source_v, Wq, bq, Wk, bk, Wv, bv,
                        g_t, b_t, g_k, b_k, g_v, b_v)
    nc = _get_module()
    res = run_bass_kernel_spmd(nc, in_maps, core_ids=list(range(B)),
                               trace=bool(int(os.environ.get("KERNEL_TRACE", "0"))))
    out = np.stack([res.results[b]["out"] for b in range(B)], axis=0)
    kernel.last_results = res
    return out
